# revision 41
# baseline (speedup 1.0000x reference)
"""Causal multi-head self-attention (B=2, L=2048, E=1024, H=16) on 8 trn2 cores.

Sharding: core c handles batch b = c//4 and head-group g = c%4 (4 heads each).
Per core the whole attention pipeline runs in a "transposed" layout so no
on-device transposes are needed:

  - host ships xT = query[b].T (contraction dim E on partitions)
  - QT/KT computed as [e_out, L]   (projection matmuls, 2 head-pairs of 128)
  - V computed as [L, e_out]       (native layout, k on partitions for AV)
  - S_T[k, q] = KT.T @ QT          (row-packed: 2 heads concurrently via
                                    tile_position row groups 0-63 / 64-127)
  - P_T = exp(S_T) (no max-sub needed: |scores| <= ~5), causal mask applied
    multiplicatively on diagonal tiles
  - ctxT_unnorm = V_ext.T @ P_T with V_ext = [V | ones] (M=65): row 64 gives
    the softmax denominator s[q] for free
  - ctxT = ctxT_unnorm * (1/s) broadcast along free dim (DMA broadcast)
  - y_partial = ctxT.T @ woT      (full K=128 contraction, head pairs packed)

Host sums the 4 per-head-group partial outputs per batch and adds w_o_b.
All matmuls run as float32r (TF32-like, full PE rate at free dim >= 256).
"""

import numpy as np

import concourse.bass as bass
import concourse.mybir as mybir
import concourse.tile as tile
from concourse import bacc
from concourse.bass_utils import run_bass_kernel_spmd

F32 = mybir.dt.float32
F32R = mybir.dt.float32r
BF16 = mybir.dt.bfloat16
AF = mybir.ActivationFunctionType

B, L, E, H, DK = 2, 2048, 1024, 16, 64
NCORES, GROUPS = 8, 4
HL = 256          # local head dims per core (4 heads x 64)
HLV = 260         # V projection width: 4 heads x (64 + ones column)
QC = 512          # q-chunk (matmul free dim)
NQC = L // QC     # 4
NKT = L // 128    # 16 k-tiles
NE = E // 128     # 8 contraction tiles for projections
SCALE = 1.0 / np.sqrt(DK)


def _r(ap):
    return ap.bitcast(F32R)


def _obs(nc, src):
    """Standalone LDWEIGHTS reading a corner of `src`: makes the PE engine
    observe src's producer semaphore so later real matmuls (whose fused
    LDWEIGHTS struct has only ONE wait slot) don't need that wait. Writes
    nothing (fp32r matmuls reload their own weights), so no WAR deps."""
    nc.tensor.ldweights(weights=src[0:1, 0:2].bitcast(mybir.dt.bfloat16))


def _emit(nc, tc, t):
    import contextlib
    from collections import deque

    ctx = contextlib.ExitStack()
    with ctx:
        persist = ctx.enter_context(tc.tile_pool(name="persist", bufs=1))

        qt = persist.tile([128, 4, L], BF16, tag="qt")
        kt_sb = persist.tile([128, 2, L], BF16, tag="kt")
        v_sb = persist.tile([128, NKT, 4, 128], BF16, tag="v")
        ctxt = persist.tile([128, 2, L], BF16, tag="ctxt")
        mask = persist.tile([128, 896], BF16, tag="mask")
        wq = persist.tile([128, NE, HL], BF16, tag="wq")
        wk = persist.tile([128, NE, HL], BF16, tag="wk")
        wv = persist.tile([128, NE, HLV], BF16, tag="wv")
        wo = persist.tile([128, 2, E], BF16, tag="wo")
        bq = persist.tile([128, 2], F32, tag="bq")
        bk = persist.tile([128, 2], F32, tag="bk")
        bv = persist.tile([128, HLV], F32, tag="bv")

        phase = contextlib.ExitStack()
        xpool = phase.enter_context(tc.tile_pool(name="xpool", bufs=2))
        psproj = phase.enter_context(
            tc.tile_pool(name="psproj", bufs=2, space="PSUM")
        )
        ppool = phase.enter_context(tc.tile_pool(name="ppool", bufs=6))
        pss = phase.enter_context(tc.tile_pool(name="pss", bufs=2, space="PSUM"))
        psav = phase.enter_context(tc.tile_pool(name="psav", bufs=2, space="PSUM"))
        npool = phase.enter_context(tc.tile_pool(name="npool", bufs=4))
        rdpool = phase.enter_context(
            tc.tile_pool(name="rdpool", bufs=1, space="DRAM")
        )
        rdram = rdpool.tile([16, QC], F32, tag="rdram")
        ypool = phase.enter_context(tc.tile_pool(name="ypool", bufs=4))

        # wq/wk and the first x chunk gate the first matmuls: issue them first
        for ke in range(NE):
            nc.sync.dma_start(out=wq[:, ke, :], in_=t["wq"][:, ke, :])
        nc.sync.dma_start(out=wk, in_=t["wk"][:])
        xt0 = xpool.tile([128, NE, QC], BF16, tag="xt", name="xt0")
        for ke in range(NE):
            nc.sync.dma_start(out=xt0[:, ke, :], in_=t["x"][0, :, ke, :])
        nc.sync.dma_start(out=wv, in_=t["wv"][:])
        nc.sync.dma_start(out=wo, in_=t["wo"][:])
        nc.sync.dma_start(out=mask, in_=t["mask"][:])
        nc.sync.dma_start(out=bq, in_=t["bq"][:])
        nc.sync.dma_start(out=bk, in_=t["bk"][:])
        nc.sync.dma_start(out=bv, in_=t["bv"][:])
        # zero-fill: qt holds each head zero-padded to K=128 (head 2i in rows
        # 0-63 of slot 2i, head 2i+1 in rows 64-127 of slot 2i+1); v_sb cols
        # 65-127 are zero so AV matmuls present a full M=128 stationary.
        # Full-array matmuls keep the PE HAM activity monitor at K=8/8 --
        # half-array ones get the clock throttled to 1.2 GHz.
        nc.vector.memset(qt, 0.0)
        nc.vector.memset(v_sb, 0.0)

        def proj_group(qc, xt, gi):
            """One projection accumulation group (8 matmuls + copy-out).
            gi 0-1: Q head-pair gi; gi 2-3: K head-pair gi-2; gi 4-7: V
            l-tile gi-4."""
            qsl = slice(qc * QC, (qc + 1) * QC)
            if gi < 4:
                w_sb, b_sb, is_q, pt = (
                    (wq, bq, True, gi) if gi < 2 else (wk, bk, False, gi - 2)
                )
                ps = psproj.tile([128, QC], F32, tag="ps", name=f"ps{qc}{gi}")
                for ke in range(NE):
                    if gi == 0:
                        # absorb the per-slice x/wq DMA waits (fused-LDWEIGHTS
                        # matmuls have a single wait slot)
                        _obs(nc, xt[:, ke, :])
                        if qc == 0:
                            _obs(nc, wq[:, ke, :])
                    nc.tensor.matmul(
                        ps,
                        lhsT=w_sb[:, ke, pt * 128 : (pt + 1) * 128],
                        rhs=xt[:, ke, :],
                        start=(ke == 0),
                        stop=(ke == NE - 1),
                    )
                if is_q:
                    nc.vector.tensor_scalar_add(
                        out=qt[0:DK, 2 * pt, qsl],
                        in0=ps[0:DK, :],
                        scalar1=b_sb[0:DK, pt : pt + 1],
                    )
                    nc.vector.tensor_scalar_add(
                        out=qt[DK:128, 2 * pt + 1, qsl],
                        in0=ps[DK:128, :],
                        scalar1=b_sb[DK:128, pt : pt + 1],
                    )
                else:
                    nc.vector.tensor_scalar_add(
                        out=kt_sb[:, pt, qsl],
                        in0=ps,
                        scalar1=b_sb[:, pt : pt + 1],
                    )
            else:
                lt4 = gi - 4
                lt = qc * 4 + lt4
                ps = psproj.tile([128, QC], F32, tag="ps", name=f"psv{lt}")
                psv = ps[:, 0:HLV]
                for ke in range(NE):
                    nc.tensor.matmul(
                        psv,
                        lhsT=xt[:, ke, lt4 * 128 : (lt4 + 1) * 128],
                        rhs=wv[:, ke, :],
                        start=(ke == 0),
                        stop=(ke == NE - 1),
                    )
                nc.vector.tensor_add(
                    out=v_sb[:, lt, :, 0 : DK + 1],
                    in0=psv.rearrange("p (h d) -> p h d", d=DK + 1),
                    in1=bv[:].rearrange("p (h d) -> p h d", d=DK + 1),
                )

        def proj_fills(qc):
            box = {}

            def mk(gi):
                def f():
                    if gi == 0:
                        xt = xpool.tile(
                            [128, NE, QC], BF16, tag="xt", name=f"xt{qc}"
                        )
                        for ke in range(NE):
                            nc.sync.dma_start(
                                out=xt[:, ke, :], in_=t["x"][qc, :, ke, :]
                            )
                        box["xt"] = xt
                    proj_group(qc, box["xt"], gi)

                return f

            return [mk(gi) for gi in range(8)]

        def outproj_fills(qc):
            def mk(lt, ec):
                def f():
                    lsl = slice(lt * 128, (lt + 1) * 128)
                    esl = slice(ec * QC, (ec + 1) * QC)
                    ps = psproj.tile(
                        [128, QC], F32, tag="ps", name=f"y{lt}{ec}"
                    )
                    for pair in range(2):
                        nc.tensor.matmul(
                            ps,
                            lhsT=ctxt[:, pair, lsl],
                            rhs=wo[:, pair, esl],
                            start=(pair == 0),
                            stop=(pair == 1),
                        )
                    ysb = ypool.tile([128, QC], F32, tag="ysb", name="ysb")
                    nc.vector.tensor_copy(out=ysb, in_=ps)
                    nc.sync.dma_start(out=t["out"][lsl, esl], in_=ysb)

                return f

            return [
                mk(lt, ec)
                for lt in range(qc * 4, qc * 4 + 4)
                for ec in range(2)
            ]

        def gen_attn(pair, qc):
            """Yields after each kt-pair quantum (2 S + 1 exp + 2 AV)."""
            nkt = 4 * qc + 4
            qsl = slice(qc * QC, (qc + 1) * QC)
            avs = [
                psav.tile([128, QC], F32, tag="av", name=f"av{pair}{qc}{i}")
                for i in range(2)
            ]
            # kt descends in pairs: diagonal (partially masked) tiles FIRST so
            # AV matmuls' single LDWEIGHTS wait slot works out (diagonal AVs
            # wait on the DVE mask-multiply whose sem value subsumes all older
            # DVE writes; non-diagonal AVs wait only on the exp).
            for kt_hi in range(nkt - 1, 0, -2):
                kts = (kt_hi, kt_hi - 1)
                for hh in range(2):
                    h = pair * 2 + hh
                    sps = pss.tile([128, 2, QC], F32, tag="s", name=f"s{hh}")
                    for i, kt in enumerate(kts):
                        nc.tensor.matmul(
                            sps[:, i, :],
                            lhsT=kt_sb[:, pair, kt * 128 : (kt + 1) * 128],
                            rhs=qt[:, h, qsl],
                            start=True,
                            stop=True,
                        )
                    p_e = ppool.tile([128, 2, QC], BF16, tag="p", name="p_e")
                    nc.scalar.activation(out=p_e, in_=sps, func=AF.Exp)
                    for i, kt in enumerate(kts):
                        r = kt - 4 * qc
                        if r >= 0:
                            p_m = ppool.tile(
                                [128, QC], BF16, tag="pm", name="p_m"
                            )
                            nc.vector.tensor_mul(
                                out=p_m,
                                in0=p_e[:, i, :],
                                in1=mask[:, (3 - r) * 128 : (3 - r) * 128 + QC],
                            )
                        else:
                            p_m = p_e[:, i, :]
                        nc.tensor.matmul(
                            avs[hh],
                            lhsT=v_sb[:, kt, h, :],
                            rhs=p_m,
                            start=(kt == nkt - 1),
                            stop=(kt == 0),
                        )
                yield
            # normalize: ctxT = ctx_unnorm / s  (s in row DK of av)
            for hh in range(2):
                av = avs[hh]
                ri = (pair * NQC + qc) * 2 + hh
                rrow = npool.tile([DK + 1, QC], F32, tag="rrow", name="rrow")
                nc.vector.tensor_copy(
                    out=rrow[DK : DK + 1, :], in_=av[DK : DK + 1, :]
                )
                nc.sync.dma_start(
                    out=rdram[ri : ri + 1, :], in_=rrow[DK : DK + 1, :]
                )
                sbc = npool.tile([DK, QC], F32, tag="sbc", name="sbc")
                rsrc = rdram[ri, :]
                nc.sync.dma_start(
                    out=sbc,
                    in_=bass.AP(
                        tensor=rsrc.tensor,
                        offset=rsrc.offset,
                        ap=[[0, DK]] + [list(p) for p in rsrc.ap],
                    ),
                )
                rbc = npool.tile([DK, QC], F32, tag="rbc", name="rbc")
                nc.vector.reciprocal_approx_fast(out=rbc, in_=sbc)
                if hh == 0:
                    nc.vector.tensor_mul(
                        out=ctxt[0:DK, pair, qsl], in0=av[0:DK, :], in1=rbc
                    )
                else:
                    sc = npool.tile([DK, QC], BF16, tag="sc", name="sc")
                    nc.vector.tensor_mul(out=sc, in0=av[0:DK, :], in1=rbc)
                    nc.gpsimd.dma_start(out=ctxt[DK:128, pair, qsl], in_=sc)

        # Weave: emit proj chunk 0, then walk attention kt-pair quanta with
        # the remaining proj/outproj groups sprinkled between quanta so the
        # ACT exp stream never starves behind a monolithic block of PE work.
        # Each era fully drains its fills before the next era's attention
        # (attention qc needs proj chunks <= qc already emitted -- a later-
        # emitted producer would deadlock the PE queue).
        _obs(nc, wk[:, 0, :])
        for gi in range(8):
            if gi == 4:
                _obs(nc, wv[:, 0, :])
            proj_group(0, xt0, gi)

        eras = [
            ([(0, 0), (1, 0)], proj_fills(1), 2),
            ([(0, 1), (1, 1)], proj_fills(2), 1),
            (
                [(0, 2), (1, 2)],
                proj_fills(3) + [lambda: _obs(nc, wo[:, 0, :])],
                1,
            ),
            (
                [(0, 3), (1, 3)],
                outproj_fills(0) + outproj_fills(1) + outproj_fills(2),
                2,
            ),
        ]
        for units, fills, k in eras:
            fills = deque(fills)
            for pair, qc in units:
                for _ in gen_attn(pair, qc):
                    for _ in range(k):
                        if fills:
                            fills.popleft()()
            while fills:
                fills.popleft()()
        for f in outproj_fills(3):
            f()
        phase.close()


def build_nc():
    nc = bacc.Bacc("TRN2", target_bir_lowering=False)
    t = {
        "x": nc.dram_tensor("x", [NQC, 128, NE, QC], BF16, kind="ExternalInput")[:],
        "wq": nc.dram_tensor("wq", [128, NE, HL], BF16, kind="ExternalInput")[:],
        "wk": nc.dram_tensor("wk", [128, NE, HL], BF16, kind="ExternalInput")[:],
        "wv": nc.dram_tensor("wv", [128, NE, HLV], BF16, kind="ExternalInput")[:],
        "wo": nc.dram_tensor("wo", [128, 2, E], BF16, kind="ExternalInput")[:],
        "bq": nc.dram_tensor("bq", [128, 2], F32, kind="ExternalInput")[:],
        "bk": nc.dram_tensor("bk", [128, 2], F32, kind="ExternalInput")[:],
        "bv": nc.dram_tensor("bv", [128, HLV], F32, kind="ExternalInput")[:],
        "mask": nc.dram_tensor("mask", [128, 896], BF16, kind="ExternalInput")[:],
        "out": nc.dram_tensor("out", [L, E], F32, kind="ExternalOutput")[:],
    }
    with tile.TileContext(nc) as tc:
        _emit(nc, tc, t)
    nc.compile()
    return nc


def _dev_layout(arr, kind):
    """Host -> device data layouts (see build_nc tensor shapes). All of these
    feed bf16 matmuls, so cast to bf16 at the end."""
    import ml_dtypes

    a = np.ascontiguousarray(arr, dtype=np.float32)
    if kind == "x":  # [1024, 2048] (already transposed) -> [NQC, 128, NE, QC]
        return np.ascontiguousarray(
            a.reshape(NE, 128, NQC, QC).transpose(2, 1, 0, 3)
        ).astype(ml_dtypes.bfloat16)
    if kind == "w3":  # [1024, W] (wT) -> [128, NE, W]
        w = a.shape[1]
        return np.ascontiguousarray(
            a.reshape(NE, 128, w).transpose(1, 0, 2)
        ).astype(ml_dtypes.bfloat16)
    if kind == "wo":  # [HL, 1024] (woT) -> [128, 2, E]
        return np.ascontiguousarray(
            a.reshape(2, 128, E).transpose(1, 0, 2)
        ).astype(ml_dtypes.bfloat16)
    if kind == "b":  # [HL] -> [128, 2]
        return np.ascontiguousarray(a.reshape(2, 128).T)
    raise ValueError(kind)


def _augment_v(vT):
    """[R, 256] -> [R, 260]: per head append a 65th column (0-weights for the
    V weight matrix, 1.0 for the bias row) that makes the AV matmul emit the
    softmax row-sum. For the bias (R=1) the appended value is 1.0; for the
    weight rows it's 0 except we pass bias separately, so: weights get 0,
    the bias row gets 1."""
    r = vT.shape[0]
    v4 = vT.reshape(r, 4, DK)
    pad_val = 1.0 if r == 1 else 0.0
    pad = np.full((r, 4, 1), pad_val, np.float32)
    return np.concatenate([v4, pad], axis=2).reshape(r, HLV)


def make_in_maps(query, w_q_w, w_q_b, w_k_w, w_k_b, w_v_w, w_v_b, w_o_w, w_o_b):
    import ml_dtypes

    mask = (
        np.arange(896, dtype=np.int64)[None, :]
        >= (np.arange(128, dtype=np.int64)[:, None] + 384)
    ).astype(ml_dtypes.bfloat16)
    x_dev = [
        _dev_layout(np.asarray(query[b], np.float32).T, "x") for b in range(B)
    ]
    in_maps = []
    for c in range(NCORES):
        b, g = divmod(c, GROUPS)
        rows = slice(g * HL, (g + 1) * HL)
        in_maps.append(
            {
                "x": x_dev[b],
                "wq": _dev_layout(np.asarray(w_q_w)[rows, :].T * SCALE, "w3"),
                "wk": _dev_layout(np.asarray(w_k_w)[rows, :].T, "w3"),
                "wv": _dev_layout(_augment_v(np.asarray(w_v_w)[rows, :].T), "w3"),
                "wo": _dev_layout(np.asarray(w_o_w)[:, rows].T, "wo"),
                "bq": _dev_layout(np.asarray(w_q_b)[rows] * SCALE, "b"),
                "bk": _dev_layout(np.asarray(w_k_b)[rows], "b"),
                "bv": np.ascontiguousarray(
                    np.broadcast_to(
                        _augment_v(np.asarray(w_v_b, np.float32)[rows][None, :])[0],
                        (128, HLV),
                    )
                ),
                "mask": mask,
            }
        )
    return in_maps


_NC_CACHE = {}


def kernel(trace=False, **inputs):
    if "nc" not in _NC_CACHE:
        _NC_CACHE["nc"] = build_nc()
    nc = _NC_CACHE["nc"]
    in_maps = make_in_maps(**inputs)
    res = run_bass_kernel_spmd(
        nc,
        in_maps,
        core_ids=list(range(NCORES)),
        trace=trace,
        trace_cores=[0] if trace else None,
    )
    w_o_b = np.asarray(inputs["w_o_b"], np.float32)
    out = np.zeros((B, L, E), dtype=np.float32)
    for c in range(NCORES):
        b = c // GROUPS
        out[b] += res.results[c]["out"]
    out += w_o_b[None, None, :]
    if trace:
        return out, res
    return out


# revision 42
# speedup vs baseline: 1.1564x; 1.1564x over previous
"""Causal multi-head self-attention (B=2, L=2048, E=1024, H=16) on 8 trn2 cores.

Sharding: core c handles batch b = c//4 and head-group g = c%4 (4 heads each).
Per core the whole attention pipeline runs in a "transposed" layout so no
on-device transposes are needed:

  - host ships xT = query[b].T (contraction dim E on partitions)
  - QT/KT computed as [e_out, L]   (projection matmuls, 2 head-pairs of 128)
  - V computed as [L, e_out]       (native layout, k on partitions for AV)
  - S_T[k, q] = KT.T @ QT          (row-packed: 2 heads concurrently via
                                    tile_position row groups 0-63 / 64-127)
  - P_T = exp(S_T) (no max-sub needed: |scores| <= ~5), causal mask applied
    multiplicatively on diagonal tiles
  - ctxT_unnorm = V_ext.T @ P_T with V_ext = [V | ones] (M=65): row 64 gives
    the softmax denominator s[q] for free
  - ctxT = ctxT_unnorm * (1/s) broadcast along free dim (DMA broadcast)
  - y_partial = ctxT.T @ woT      (full K=128 contraction, head pairs packed)

Host sums the 4 per-head-group partial outputs per batch and adds w_o_b.
All matmuls run as float32r (TF32-like, full PE rate at free dim >= 256).
"""

import numpy as np

import concourse.bass as bass
import concourse.mybir as mybir
import concourse.tile as tile
from concourse import bacc
from concourse.bass_utils import run_bass_kernel_spmd

F32 = mybir.dt.float32
F32R = mybir.dt.float32r
BF16 = mybir.dt.bfloat16
AF = mybir.ActivationFunctionType

B, L, E, H, DK = 2, 2048, 1024, 16, 64
NCORES, GROUPS = 8, 4
HL = 256          # local head dims per core (4 heads x 64)
HLV = 260         # V projection width: 4 heads x (64 + ones column)
QC = 512          # q-chunk (matmul free dim)
NQC = L // QC     # 4
NKT = L // 128    # 16 k-tiles
NE = E // 128     # 8 contraction tiles for projections
SCALE = 1.0 / np.sqrt(DK)


def _r(ap):
    return ap.bitcast(F32R)


def _obs(nc, src):
    """Standalone LDWEIGHTS reading a corner of `src`: makes the PE engine
    observe src's producer semaphore so later real matmuls (whose fused
    LDWEIGHTS struct has only ONE wait slot) don't need that wait. Writes
    nothing (fp32r matmuls reload their own weights), so no WAR deps."""
    nc.tensor.ldweights(weights=src[0:1, 0:2].bitcast(mybir.dt.bfloat16))


def _emit(nc, tc, t):
    import contextlib
    from collections import deque

    ctx = contextlib.ExitStack()
    with ctx:
        persist = ctx.enter_context(tc.tile_pool(name="persist", bufs=1))

        qt = persist.tile([128, 4, L], BF16, tag="qt")
        kt_sb = persist.tile([128, 2, L], BF16, tag="kt")
        v_sb = persist.tile([128, NKT, 4, 128], BF16, tag="v")
        ctxt = persist.tile([128, 2, L], BF16, tag="ctxt")
        mask = persist.tile([128, 896], BF16, tag="mask")
        wq = persist.tile([128, NE, HL], BF16, tag="wq")
        wk = persist.tile([128, NE, HL], BF16, tag="wk")
        wv = persist.tile([128, NE, HLV], BF16, tag="wv")
        wo = persist.tile([128, 2, E], BF16, tag="wo")
        bq = persist.tile([128, 2], F32, tag="bq")
        bk = persist.tile([128, 2], F32, tag="bk")
        bv = persist.tile([128, HLV], F32, tag="bv")

        phase = contextlib.ExitStack()
        xpool = phase.enter_context(tc.tile_pool(name="xpool", bufs=2))
        psproj = phase.enter_context(
            tc.tile_pool(name="psproj", bufs=2, space="PSUM")
        )
        ppool = phase.enter_context(tc.tile_pool(name="ppool", bufs=6))
        pss = phase.enter_context(tc.tile_pool(name="pss", bufs=2, space="PSUM"))
        psav = phase.enter_context(tc.tile_pool(name="psav", bufs=2, space="PSUM"))
        npool = phase.enter_context(tc.tile_pool(name="npool", bufs=4))
        rdpool = phase.enter_context(
            tc.tile_pool(name="rdpool", bufs=1, space="DRAM")
        )
        rdram = rdpool.tile([16, QC], F32, tag="rdram")
        ypool = phase.enter_context(tc.tile_pool(name="ypool", bufs=4))

        # wq/wk and the first x chunk gate the first matmuls: issue them first
        for ke in range(NE):
            nc.sync.dma_start(out=wq[:, ke, :], in_=t["wq"][:, ke, :])
        nc.sync.dma_start(out=wk, in_=t["wk"][:])
        xt0 = xpool.tile([128, NE, QC], BF16, tag="xt", name="xt0")
        for ke in range(NE):
            nc.sync.dma_start(out=xt0[:, ke, :], in_=t["x"][0, :, ke, :])
        nc.sync.dma_start(out=wv, in_=t["wv"][:])
        nc.sync.dma_start(out=wo, in_=t["wo"][:])
        nc.sync.dma_start(out=mask, in_=t["mask"][:])
        nc.sync.dma_start(out=bq, in_=t["bq"][:])
        nc.sync.dma_start(out=bk, in_=t["bk"][:])
        nc.sync.dma_start(out=bv, in_=t["bv"][:])
        # zero-fill: qt holds each head zero-padded to K=128 (head 2i in rows
        # 0-63 of slot 2i, head 2i+1 in rows 64-127 of slot 2i+1); v_sb cols
        # 65-127 are zero so AV matmuls present a full M=128 stationary.
        # Full-array matmuls keep the PE HAM activity monitor at K=8/8 --
        # half-array ones get the clock throttled to 1.2 GHz.
        nc.vector.memset(qt, 0.0)
        nc.vector.memset(v_sb, 0.0)

        def proj_group(qc, xt, gi):
            """One projection accumulation group (8 matmuls + copy-out).
            gi 0-1: Q head-pair gi; gi 2-3: K head-pair gi-2; gi 4-7: V
            l-tile gi-4."""
            qsl = slice(qc * QC, (qc + 1) * QC)
            if gi < 4:
                w_sb, b_sb, is_q, pt = (
                    (wq, bq, True, gi) if gi < 2 else (wk, bk, False, gi - 2)
                )
                ps = psproj.tile([128, QC], F32, tag="ps", name=f"ps{qc}{gi}")
                for ke in range(NE):
                    if gi == 0:
                        # absorb the per-slice x/wq DMA waits (fused-LDWEIGHTS
                        # matmuls have a single wait slot)
                        _obs(nc, xt[:, ke, :])
                        if qc == 0:
                            _obs(nc, wq[:, ke, :])
                    nc.tensor.matmul(
                        ps,
                        lhsT=w_sb[:, ke, pt * 128 : (pt + 1) * 128],
                        rhs=xt[:, ke, :],
                        start=(ke == 0),
                        stop=(ke == NE - 1),
                    )
                if is_q:
                    nc.vector.tensor_scalar_add(
                        out=qt[0:DK, 2 * pt, qsl],
                        in0=ps[0:DK, :],
                        scalar1=b_sb[0:DK, pt : pt + 1],
                    )
                    nc.vector.tensor_scalar_add(
                        out=qt[DK:128, 2 * pt + 1, qsl],
                        in0=ps[DK:128, :],
                        scalar1=b_sb[DK:128, pt : pt + 1],
                    )
                else:
                    nc.vector.tensor_scalar_add(
                        out=kt_sb[:, pt, qsl],
                        in0=ps,
                        scalar1=b_sb[:, pt : pt + 1],
                    )
            else:
                lt4 = gi - 4
                lt = qc * 4 + lt4
                ps = psproj.tile([128, QC], F32, tag="ps", name=f"psv{lt}")
                psv = ps[:, 0:HLV]
                for ke in range(NE):
                    nc.tensor.matmul(
                        psv,
                        lhsT=xt[:, ke, lt4 * 128 : (lt4 + 1) * 128],
                        rhs=wv[:, ke, :],
                        start=(ke == 0),
                        stop=(ke == NE - 1),
                    )
                nc.vector.tensor_add(
                    out=v_sb[:, lt, :, 0 : DK + 1],
                    in0=psv.rearrange("p (h d) -> p h d", d=DK + 1),
                    in1=bv[:].rearrange("p (h d) -> p h d", d=DK + 1),
                )

        def proj_fills(qc):
            box = {}

            def mk(gi):
                def f():
                    if gi == 0:
                        xt = xpool.tile(
                            [128, NE, QC], BF16, tag="xt", name=f"xt{qc}"
                        )
                        for ke in range(NE):
                            nc.sync.dma_start(
                                out=xt[:, ke, :], in_=t["x"][qc, :, ke, :]
                            )
                        box["xt"] = xt
                    proj_group(qc, box["xt"], gi)

                return f

            return [mk(gi) for gi in range(8)]

        def outproj_fills(qc):
            def mk(lt, ec):
                def f():
                    lsl = slice(lt * 128, (lt + 1) * 128)
                    esl = slice(ec * QC, (ec + 1) * QC)
                    ps = psproj.tile(
                        [128, QC], F32, tag="ps", name=f"y{lt}{ec}"
                    )
                    for pair in range(2):
                        nc.tensor.matmul(
                            ps,
                            lhsT=ctxt[:, pair, lsl],
                            rhs=wo[:, pair, esl],
                            start=(pair == 0),
                            stop=(pair == 1),
                        )
                    ysb = ypool.tile([128, QC], F32, tag="ysb", name="ysb")
                    nc.vector.tensor_copy(out=ysb, in_=ps)
                    nc.sync.dma_start(out=t["out"][lsl, esl], in_=ysb)

                return f

            return [
                mk(lt, ec)
                for lt in range(qc * 4, qc * 4 + 4)
                for ec in range(2)
            ]

        def gen_attn(pair, qc):
            """Yields after each kt-pair quantum (2 S + 1 exp + 2 AV)."""
            nkt = 4 * qc + 4
            qsl = slice(qc * QC, (qc + 1) * QC)
            avs = [
                psav.tile([128, QC], F32, tag="av", name=f"av{pair}{qc}{i}")
                for i in range(2)
            ]
            # kt descends in pairs: diagonal (partially masked) tiles FIRST so
            # AV matmuls' single LDWEIGHTS wait slot works out (diagonal AVs
            # wait on the DVE mask-multiply whose sem value subsumes all older
            # DVE writes; non-diagonal AVs wait only on the exp).
            for kt_hi in range(nkt - 1, 0, -2):
                kts = (kt_hi, kt_hi - 1)
                for hh in range(2):
                    h = pair * 2 + hh
                    sps = pss.tile([128, 2, QC], F32, tag="s", name=f"s{hh}")
                    for i, kt in enumerate(kts):
                        nc.tensor.matmul(
                            sps[:, i, :],
                            lhsT=kt_sb[:, pair, kt * 128 : (kt + 1) * 128],
                            rhs=qt[:, h, qsl],
                            start=True,
                            stop=True,
                        )
                    p_e = ppool.tile([128, 2, QC], BF16, tag="p", name="p_e")
                    nc.scalar.activation(out=p_e, in_=sps, func=AF.Exp)
                    for i, kt in enumerate(kts):
                        r = kt - 4 * qc
                        if r >= 0:
                            p_m = ppool.tile(
                                [128, QC], BF16, tag="pm", name="p_m"
                            )
                            nc.vector.tensor_mul(
                                out=p_m,
                                in0=p_e[:, i, :],
                                in1=mask[:, (3 - r) * 128 : (3 - r) * 128 + QC],
                            )
                        else:
                            p_m = p_e[:, i, :]
                        nc.tensor.matmul(
                            avs[hh],
                            lhsT=v_sb[:, kt, h, :],
                            rhs=p_m,
                            start=(kt == nkt - 1),
                            stop=(kt == 0),
                        )
                yield
            # normalize: ctxT = ctx_unnorm / s  (s in row DK of av)
            for hh in range(2):
                av = avs[hh]
                ri = (pair * NQC + qc) * 2 + hh
                rrow = npool.tile([DK + 1, QC], F32, tag="rrow", name="rrow")
                nc.vector.tensor_copy(
                    out=rrow[DK : DK + 1, :], in_=av[DK : DK + 1, :]
                )
                nc.sync.dma_start(
                    out=rdram[ri : ri + 1, :], in_=rrow[DK : DK + 1, :]
                )
                sbc = npool.tile([DK, QC], F32, tag="sbc", name="sbc")
                rsrc = rdram[ri, :]
                nc.sync.dma_start(
                    out=sbc,
                    in_=bass.AP(
                        tensor=rsrc.tensor,
                        offset=rsrc.offset,
                        ap=[[0, DK]] + [list(p) for p in rsrc.ap],
                    ),
                )
                rbc = npool.tile([DK, QC], F32, tag="rbc", name="rbc")
                nc.vector.reciprocal_approx_fast(out=rbc, in_=sbc)
                if hh == 0:
                    nc.vector.tensor_mul(
                        out=ctxt[0:DK, pair, qsl], in0=av[0:DK, :], in1=rbc
                    )
                else:
                    sc = npool.tile([DK, QC], BF16, tag="sc", name="sc")
                    nc.vector.tensor_mul(out=sc, in0=av[0:DK, :], in1=rbc)
                    nc.gpsimd.dma_start(out=ctxt[DK:128, pair, qsl], in_=sc)

        # Weave: emit proj chunk 0, then walk attention kt-pair quanta with
        # the remaining proj/outproj groups sprinkled between quanta so the
        # ACT exp stream never starves behind a monolithic block of PE work.
        # Each era fully drains its fills before the next era's attention
        # (attention qc needs proj chunks <= qc already emitted -- a later-
        # emitted producer would deadlock the PE queue).
        _obs(nc, wk[:, 0, :])
        for gi in range(8):
            if gi == 4:
                _obs(nc, wv[:, 0, :])
            proj_group(0, xt0, gi)

        eras = [
            ([(0, 0), (1, 0)], proj_fills(1), 2),
            ([(0, 1), (1, 1)], proj_fills(2), 1),
            (
                [(0, 2), (1, 2)],
                proj_fills(3) + [lambda: _obs(nc, wo[:, 0, :])] + outproj_fills(0),
                2,
            ),
            ([(0, 3), (1, 3)], outproj_fills(1) + outproj_fills(2), 1),
        ]
        for units, fills, k in eras:
            fills = deque(fills)
            for pair, qc in units:
                for _ in gen_attn(pair, qc):
                    for _ in range(k):
                        if fills:
                            fills.popleft()()
            while fills:
                fills.popleft()()
        for f in outproj_fills(3):
            f()
        phase.close()


def build_nc():
    nc = bacc.Bacc("TRN2", target_bir_lowering=False)
    t = {
        "x": nc.dram_tensor("x", [NQC, 128, NE, QC], BF16, kind="ExternalInput")[:],
        "wq": nc.dram_tensor("wq", [128, NE, HL], BF16, kind="ExternalInput")[:],
        "wk": nc.dram_tensor("wk", [128, NE, HL], BF16, kind="ExternalInput")[:],
        "wv": nc.dram_tensor("wv", [128, NE, HLV], BF16, kind="ExternalInput")[:],
        "wo": nc.dram_tensor("wo", [128, 2, E], BF16, kind="ExternalInput")[:],
        "bq": nc.dram_tensor("bq", [128, 2], F32, kind="ExternalInput")[:],
        "bk": nc.dram_tensor("bk", [128, 2], F32, kind="ExternalInput")[:],
        "bv": nc.dram_tensor("bv", [128, HLV], F32, kind="ExternalInput")[:],
        "mask": nc.dram_tensor("mask", [128, 896], BF16, kind="ExternalInput")[:],
        "out": nc.dram_tensor("out", [L, E], F32, kind="ExternalOutput")[:],
    }
    with tile.TileContext(nc) as tc:
        _emit(nc, tc, t)
    nc.compile()
    return nc


def _dev_layout(arr, kind):
    """Host -> device data layouts (see build_nc tensor shapes). All of these
    feed bf16 matmuls, so cast to bf16 at the end."""
    import ml_dtypes

    a = np.ascontiguousarray(arr, dtype=np.float32)
    if kind == "x":  # [1024, 2048] (already transposed) -> [NQC, 128, NE, QC]
        return np.ascontiguousarray(
            a.reshape(NE, 128, NQC, QC).transpose(2, 1, 0, 3)
        ).astype(ml_dtypes.bfloat16)
    if kind == "w3":  # [1024, W] (wT) -> [128, NE, W]
        w = a.shape[1]
        return np.ascontiguousarray(
            a.reshape(NE, 128, w).transpose(1, 0, 2)
        ).astype(ml_dtypes.bfloat16)
    if kind == "wo":  # [HL, 1024] (woT) -> [128, 2, E]
        return np.ascontiguousarray(
            a.reshape(2, 128, E).transpose(1, 0, 2)
        ).astype(ml_dtypes.bfloat16)
    if kind == "b":  # [HL] -> [128, 2]
        return np.ascontiguousarray(a.reshape(2, 128).T)
    raise ValueError(kind)


def _augment_v(vT):
    """[R, 256] -> [R, 260]: per head append a 65th column (0-weights for the
    V weight matrix, 1.0 for the bias row) that makes the AV matmul emit the
    softmax row-sum. For the bias (R=1) the appended value is 1.0; for the
    weight rows it's 0 except we pass bias separately, so: weights get 0,
    the bias row gets 1."""
    r = vT.shape[0]
    v4 = vT.reshape(r, 4, DK)
    pad_val = 1.0 if r == 1 else 0.0
    pad = np.full((r, 4, 1), pad_val, np.float32)
    return np.concatenate([v4, pad], axis=2).reshape(r, HLV)


def make_in_maps(query, w_q_w, w_q_b, w_k_w, w_k_b, w_v_w, w_v_b, w_o_w, w_o_b):
    import ml_dtypes

    mask = (
        np.arange(896, dtype=np.int64)[None, :]
        >= (np.arange(128, dtype=np.int64)[:, None] + 384)
    ).astype(ml_dtypes.bfloat16)
    x_dev = [
        _dev_layout(np.asarray(query[b], np.float32).T, "x") for b in range(B)
    ]
    in_maps = []
    for c in range(NCORES):
        b, g = divmod(c, GROUPS)
        rows = slice(g * HL, (g + 1) * HL)
        in_maps.append(
            {
                "x": x_dev[b],
                "wq": _dev_layout(np.asarray(w_q_w)[rows, :].T * SCALE, "w3"),
                "wk": _dev_layout(np.asarray(w_k_w)[rows, :].T, "w3"),
                "wv": _dev_layout(_augment_v(np.asarray(w_v_w)[rows, :].T), "w3"),
                "wo": _dev_layout(np.asarray(w_o_w)[:, rows].T, "wo"),
                "bq": _dev_layout(np.asarray(w_q_b)[rows] * SCALE, "b"),
                "bk": _dev_layout(np.asarray(w_k_b)[rows], "b"),
                "bv": np.ascontiguousarray(
                    np.broadcast_to(
                        _augment_v(np.asarray(w_v_b, np.float32)[rows][None, :])[0],
                        (128, HLV),
                    )
                ),
                "mask": mask,
            }
        )
    return in_maps


_NC_CACHE = {}


def kernel(trace=False, **inputs):
    if "nc" not in _NC_CACHE:
        _NC_CACHE["nc"] = build_nc()
    nc = _NC_CACHE["nc"]
    in_maps = make_in_maps(**inputs)
    res = run_bass_kernel_spmd(
        nc,
        in_maps,
        core_ids=list(range(NCORES)),
        trace=trace,
        trace_cores=[0] if trace else None,
    )
    w_o_b = np.asarray(inputs["w_o_b"], np.float32)
    out = np.zeros((B, L, E), dtype=np.float32)
    for c in range(NCORES):
        b = c // GROUPS
        out[b] += res.results[c]["out"]
    out += w_o_b[None, None, :]
    if trace:
        return out, res
    return out
